# revision 1
# baseline (speedup 1.0000x reference)
"""DiffLogicLayer forward on 8 TRN2 NeuronCores.

Math: every one of the 16 soft logic ops is affine in {1, a, b, a*b}, so
    out[n, o] = C0[o] + C1[o]*a + C2[o]*b + C3[o]*a*b
with a = x[n, conn_a[o]], b = x[n, conn_b[o]] and C = softmax(weights) @ M
for the constant 16x4 matrix M of op coefficients.

Sharding: out_dim (gate axis) split 8 ways; each core owns 1024 gates and
the full batch. Host supplies xT = x.T so that "column of x" = contiguous
16 KiB row; per 128-gate slot the core dma_gathers the two operand rows
(gates land on partitions), computes
    u = C3*a + C2   (ACT, per-partition scale/bias)
    w = C1*a + C0   (ACT)
    v = u * b       (DVE)
    out = v + w     (DVE)
and DMAs the [128, 4096] slot to outT. C0..C3 are computed on-device from
the weights shard (exp -> strided-window reduces -> signed sums -> * 1/Z).
Host transposes/concats the per-core outT shards into the full output.
"""

import numpy as np
from contextlib import ExitStack

import concourse.bacc as bacc
import concourse.mybir as mybir
import concourse.tile as tile
from concourse.bass_utils import run_bass_kernel_spmd

N_CORES = 8
BATCH, IN_DIM, OUT_DIM = 4096, 4096, 8192
GPC = OUT_DIM // N_CORES          # gates per core = 1024
SLOTS = GPC // 128                # 128-gate slots per core = 8
F32 = mybir.dt.float32

_compiled = {}


def _build_nc():
    nc = bacc.Bacc("TRN2", target_bir_lowering=False, debug=False,
                   num_devices=N_CORES)
    xT = nc.dram_tensor("xT", [IN_DIM, BATCH], F32, kind="ExternalInput")
    ia_d = nc.dram_tensor("ia", [128, SLOTS * 8], mybir.dt.int16,
                          kind="ExternalInput")
    ib_d = nc.dram_tensor("ib", [128, SLOTS * 8], mybir.dt.int16,
                          kind="ExternalInput")
    wt = nc.dram_tensor("wt", [GPC, 16], F32, kind="ExternalInput")
    outT = nc.dram_tensor("outT", [GPC, BATCH], F32, kind="ExternalOutput")

    with tile.TileContext(nc) as tc, ExitStack() as ctx:
        const = ctx.enter_context(tc.tile_pool(name="const", bufs=1))
        pa = ctx.enter_context(tc.tile_pool(name="a", bufs=2))
        pb = ctx.enter_context(tc.tile_pool(name="b", bufs=2))
        pu = ctx.enter_context(tc.tile_pool(name="u", bufs=2))
        pw = ctx.enter_context(tc.tile_pool(name="w", bufs=2))
        po = ctx.enter_context(tc.tile_pool(name="o", bufs=2))

        # ---- index tiles (already wrapped per slot on host) ----
        ia = const.tile([128, SLOTS * 8], mybir.dt.int16, tag="ia")
        ib = const.tile([128, SLOTS * 8], mybir.dt.int16, tag="ib")
        nc.sync.dma_start(ia[:], ia_d.ap()[:])
        nc.sync.dma_start(ib[:], ib_d.ap()[:])

        # ---- per-gate coefficients from weights ----
        W = const.tile([128, SLOTS, 16], F32, tag="W")
        nc.sync.dma_start(W[:], wt.ap().rearrange("(s p) i -> p s i", p=128))
        E = const.tile([128, SLOTS, 16], F32, tag="E")
        nc.scalar.activation(E[:], W[:], mybir.ActivationFunctionType.Exp)

        def red(lo, hi, tag):
            t = const.tile([128, SLOTS], F32, tag=tag)
            nc.vector.tensor_reduce(t[:], E[:, :, lo:hi],
                                    mybir.AxisListType.X, mybir.AluOpType.add)
            return t

        Z = red(0, 16, "Z")
        R = const.tile([128, SLOTS], F32, tag="R")
        nc.vector.reciprocal(R[:], Z[:])

        # C0 = e8..e15
        C0 = red(8, 16, "C0")
        # C1 = (e2+e3) + (e6+e7) - (e8+e9) - (e12+e13)
        P23, P67, P89, P1213 = (red(2, 4, "P23"), red(6, 8, "P67"),
                                red(8, 10, "P89"), red(12, 14, "P1213"))
        C1 = const.tile([128, SLOTS], F32, tag="C1")
        nc.vector.tensor_add(C1[:], P23[:], P67[:])
        nc.vector.tensor_sub(C1[:], C1[:], P89[:])
        nc.vector.tensor_sub(C1[:], C1[:], P1213[:])
        # C2 = (e4..e7) - (e8+e9) - (e10+e11)
        P4567, P1011 = red(4, 8, "P4567"), red(10, 12, "P1011")
        C2 = const.tile([128, SLOTS], F32, tag="C2")
        nc.vector.tensor_sub(C2[:], P4567[:], P89[:])
        nc.vector.tensor_sub(C2[:], C2[:], P1011[:])
        # C3 = e1 - e2 - e4 - 2e6 - e7 + e8 + 2e9 + e11 + e13 - e14
        #    = (e1+e8+e11+e13) + 2(e9-e6) - (e2+e4+e7+e14)
        def sl(i):
            return E[:, :, i]

        C3 = const.tile([128, SLOTS], F32, tag="C3")
        t1 = const.tile([128, SLOTS], F32, tag="t1")
        nc.vector.tensor_add(C3[:], sl(1), sl(8))
        nc.vector.tensor_add(C3[:], C3[:], sl(11))
        nc.vector.tensor_add(C3[:], C3[:], sl(13))
        nc.vector.tensor_sub(t1[:], sl(9), sl(6))
        nc.vector.tensor_add(C3[:], C3[:], t1[:])
        nc.vector.tensor_add(C3[:], C3[:], t1[:])
        nc.vector.tensor_add(t1[:], sl(2), sl(4))
        nc.vector.tensor_add(t1[:], t1[:], sl(7))
        nc.vector.tensor_add(t1[:], t1[:], sl(14))
        nc.vector.tensor_sub(C3[:], C3[:], t1[:])
        # normalize by softmax denominator
        for C in (C0, C1, C2, C3):
            nc.vector.tensor_mul(C[:], C[:], R[:])

        # ---- main loop over 128-gate slots ----
        for s in range(SLOTS):
            a = pa.tile([128, 1, BATCH], F32, tag="a")
            nc.gpsimd.dma_gather(a[:], xT.ap()[:], ia[:, s * 8:(s + 1) * 8],
                                 128, 128, BATCH)
            b = pb.tile([128, 1, BATCH], F32, tag="b")
            nc.gpsimd.dma_gather(b[:], xT.ap()[:], ib[:, s * 8:(s + 1) * 8],
                                 128, 128, BATCH)
            a2, b2 = a[:, 0, :], b[:, 0, :]
            u = pu.tile([128, BATCH], F32, tag="u")
            nc.scalar.activation(u[:], a2, mybir.ActivationFunctionType.Identity,
                                 bias=C2[:, s : s + 1], scale=C3[:, s : s + 1])
            w = pw.tile([128, BATCH], F32, tag="w")
            nc.scalar.activation(w[:], a2, mybir.ActivationFunctionType.Identity,
                                 bias=C0[:, s : s + 1], scale=C1[:, s : s + 1])
            nc.vector.tensor_mul(u[:], u[:], b2)
            o = po.tile([128, BATCH], F32, tag="o")
            nc.vector.tensor_add(o[:], u[:], w[:])
            nc.sync.dma_start(outT.ap()[s * 128:(s + 1) * 128, :], o[:])

    nc.compile()
    return nc


def _wrap_idx(conn_shard: np.ndarray) -> np.ndarray:
    """Per-slot SWDGE wrapping: slot s covers list positions s*128..s*128+127;
    within a slot, position i sits at partition i%16, free slot i//16,
    replicated across the 8 Q7 cores (partition blocks of 16)."""
    w = np.empty((128, SLOTS * 8), np.int16)
    for s in range(SLOTS):
        blk = conn_shard[s * 128:(s + 1) * 128].reshape(8, 16).T
        w[:, s * 8:(s + 1) * 8] = np.tile(blk, (8, 1))
    return w


def make_in_maps(x, weights, conn_a, conn_b):
    x = np.asarray(x, dtype=np.float32)
    weights = np.asarray(weights, dtype=np.float32)
    ca = np.asarray(conn_a).astype(np.int64)
    cb = np.asarray(conn_b).astype(np.int64)
    xT = np.ascontiguousarray(x.T)
    in_maps = []
    for c in range(N_CORES):
        g0, g1 = c * GPC, (c + 1) * GPC
        in_maps.append({
            "xT": xT,
            "ia": _wrap_idx(ca[g0:g1].astype(np.int16)),
            "ib": _wrap_idx(cb[g0:g1].astype(np.int16)),
            "wt": np.ascontiguousarray(weights[g0:g1]),
        })
    return in_maps


def get_nc():
    if "nc" not in _compiled:
        _compiled["nc"] = _build_nc()
    return _compiled["nc"]


def assemble_out(results) -> np.ndarray:
    out = np.empty((BATCH, OUT_DIM), np.float32)
    for c in range(N_CORES):
        out[:, c * GPC:(c + 1) * GPC] = results[c]["outT"].T
    return out


def kernel(x, weights, conn_a, conn_b) -> np.ndarray:
    nc = get_nc()
    in_maps = make_in_maps(x, weights, conn_a, conn_b)
    res = run_bass_kernel_spmd(nc, in_maps, core_ids=list(range(N_CORES)))
    return assemble_out(res.results)



# revision 5
# speedup vs baseline: 1.3559x; 1.3559x over previous
"""DiffLogicLayer forward on 8 TRN2 NeuronCores — batch-sharded, bf16.

Math: every one of the 16 soft logic ops is affine in {1, a, b, a*b}, so
    out[n, o] = C0[o] + C1[o]*a + C2[o]*b + C3[o]*a*b
with a = x[n, conn_a[o]], b = x[n, conn_b[o]] and C = softmax(weights) @ M
for the constant 16x4 matrix M of op coefficients (host-precomputed; it is
O(out_dim) work).

Sharding: batch split 8 ways; each core owns 512 batch rows and all 8192
gates. Host supplies xc = x[c*512:(c+1)*512].T as bf16 so each in_dim index
is one contiguous 1 KiB row. Gates are processed in 8 chunks of 1024: two
dma_gathers pull the operand rows for 1024 gates (gates land on partitions,
8 slots of 128 on the free axis), then per 128-gate slot
    u = C3*a + C2   (DVE tensor_scalar, per-partition scalars)
    w = C1*a + C0   (ACT, per-partition scale/bias)
    u = u * b       (DVE)
    o = u + w       (DVE)
all in bf16, and the [128, 8, 512] chunk is DMAd contiguously to DRAM.
Host unscrambles the [8, 128, 8, 512] per-core output and casts to f32.
"""

import numpy as np
from contextlib import ExitStack

import concourse.bacc as bacc
import concourse.mybir as mybir
import concourse.tile as tile
from concourse.bass_utils import run_bass_kernel_spmd

N_CORES = 8
BATCH, IN_DIM, OUT_DIM = 4096, 4096, 8192
BPC = BATCH // N_CORES            # batch rows per core = 512
NSLOT = OUT_DIM // 128            # 128-gate slots per core = 64
NCHUNK = 8                        # gather chunks (1024 gates each)
SPC = NSLOT // NCHUNK             # slots per chunk = 8
F32 = mybir.dt.float32
BF16 = mybir.dt.bfloat16
I16 = mybir.dt.int16
NP_BF16 = mybir.dt.np(BF16)

# coefficient matrix: op i -> (c0, c1, c2, c3) with value c0 + c1*a + c2*b
# + c3*a*b; rows follow the reference's 16-op ordering.
_OP2AFF = np.array([
    [0, 0, 0, 0],     # false
    [0, 0, 0, 1],     # a and b
    [0, 1, 0, -1],    # a and not b
    [0, 1, 0, 0],     # a
    [0, 0, 1, -1],    # not a and b
    [0, 0, 1, 0],     # b
    [0, 1, 1, -2],    # xor
    [0, 1, 1, -1],    # or
    [1, -1, -1, 1],   # nor
    [1, -1, -1, 2],   # xnor
    [1, 0, -1, 0],    # not b
    [1, 0, -1, 1],    # a or not b
    [1, -1, 0, 0],    # not a
    [1, -1, 0, 1],    # not a or b
    [1, 0, 0, -1],    # nand
    [1, 0, 0, 0],     # true
], dtype=np.float32)

_compiled = {}


def _build_nc():
    nc = bacc.Bacc("TRN2", target_bir_lowering=False, debug=False,
                   num_devices=N_CORES)
    xc = nc.dram_tensor("xc", [IN_DIM, BPC], BF16, kind="ExternalInput")
    ia_d = nc.dram_tensor("ia", [128, NCHUNK * 64], I16, kind="ExternalInput")
    ib_d = nc.dram_tensor("ib", [128, NCHUNK * 64], I16, kind="ExternalInput")
    cf_d = nc.dram_tensor("cf", [4, 128, NSLOT], F32, kind="ExternalInput")
    outd = nc.dram_tensor("outd", [NCHUNK, 128, SPC, BPC], BF16,
                          kind="ExternalOutput")

    with tile.TileContext(nc) as tc, ExitStack() as ctx:
        const = ctx.enter_context(tc.tile_pool(name="const", bufs=1))
        pa = ctx.enter_context(tc.tile_pool(name="a", bufs=2))
        pb = ctx.enter_context(tc.tile_pool(name="b", bufs=2))
        pu = ctx.enter_context(tc.tile_pool(name="u", bufs=3))
        pw = ctx.enter_context(tc.tile_pool(name="w", bufs=3))
        po = ctx.enter_context(tc.tile_pool(name="o", bufs=2))

        ia = const.tile([128, NCHUNK * 64], I16, tag="ia")
        ib = const.tile([128, NCHUNK * 64], I16, tag="ib")
        nc.sync.dma_start(ia[:], ia_d.ap()[:])
        nc.sync.dma_start(ib[:], ib_d.ap()[:])
        cs = []
        for k in range(4):
            ck = const.tile([128, NSLOT], F32, tag=f"c{k}")
            nc.sync.dma_start(ck[:], cf_d.ap()[k])
            cs.append(ck)
        C0, C1, C2, C3 = cs

        for gc in range(NCHUNK):
            A = pa.tile([128, SPC, BPC], BF16, tag="A")
            nc.gpsimd.dma_gather(A[:], xc.ap()[:],
                                 ia[:, gc * 64:(gc + 1) * 64],
                                 SPC * 128, SPC * 128, BPC)
            B = pb.tile([128, SPC, BPC], BF16, tag="B")
            nc.gpsimd.dma_gather(B[:], xc.ap()[:],
                                 ib[:, gc * 64:(gc + 1) * 64],
                                 SPC * 128, SPC * 128, BPC)
            o = po.tile([128, SPC, BPC], BF16, tag="o")
            for c in range(SPC):
                s = gc * SPC + c
                a2, b2 = A[:, c, :], B[:, c, :]
                u = pu.tile([128, BPC], BF16, tag="u")
                nc.vector.tensor_scalar(u[:], a2, C3[:, s:s + 1],
                                        C2[:, s:s + 1],
                                        mybir.AluOpType.mult,
                                        mybir.AluOpType.add)
                w = pw.tile([128, BPC], BF16, tag="w")
                nc.scalar.activation(w[:], a2,
                                     mybir.ActivationFunctionType.Identity,
                                     bias=C0[:, s:s + 1],
                                     scale=C1[:, s:s + 1])
                nc.vector.tensor_mul(u[:], u[:], b2)
                nc.vector.tensor_add(o[:, c, :], u[:], w[:])
            nc.sync.dma_start(outd.ap()[gc], o[:])

    nc.compile()
    return nc


def _wrap_idx(conn: np.ndarray) -> np.ndarray:
    """SWDGE index wrapping: per 1024-gate chunk, list position i sits at
    partition i%16, free slot i//16, replicated across the 8 Q7 core
    partition blocks of 16."""
    w = np.empty((128, NCHUNK * 64), np.int16)
    for gc in range(NCHUNK):
        blk = conn[gc * 1024:(gc + 1) * 1024].reshape(64, 16).T
        w[:, gc * 64:(gc + 1) * 64] = np.tile(blk, (8, 1))
    return w


def make_in_maps(x, weights, conn_a, conn_b):
    x = np.asarray(x, dtype=np.float32)
    weights = np.asarray(weights, dtype=np.float32)
    ca = np.asarray(conn_a).astype(np.int16)
    cb = np.asarray(conn_b).astype(np.int16)
    # softmax(weights) @ affine-coefficient matrix -> [OUT_DIM, 4] f32
    e = np.exp(weights - weights.max(axis=1, keepdims=True))
    sm = e / e.sum(axis=1, keepdims=True)
    cofs = sm @ _OP2AFF                                  # [OUT_DIM, 4]
    # gate g = 128*s + p  ->  cf[k, p, s]
    cf = np.ascontiguousarray(
        cofs.reshape(NSLOT, 128, 4).transpose(2, 1, 0)).astype(np.float32)
    ia = _wrap_idx(ca)
    ib = _wrap_idx(cb)
    in_maps = []
    for c in range(N_CORES):
        xs = x[c * BPC:(c + 1) * BPC, :]                 # [512, 4096]
        xcT = np.ascontiguousarray(xs.T.astype(NP_BF16))  # [4096, 512] bf16
        in_maps.append({"xc": xcT, "ia": ia, "ib": ib, "cf": cf})
    return in_maps


def get_nc():
    if "nc" not in _compiled:
        _compiled["nc"] = _build_nc()
    return _compiled["nc"]


def assemble_out(results) -> np.ndarray:
    out = np.empty((BATCH, OUT_DIM), np.float32)
    for c in range(N_CORES):
        arr = np.asarray(results[c]["outd"])             # [8, 128, 8, 512]
        # out[n, g] with g = 1024*gc + 128*cc + p  <-  arr[gc, p, cc, n]
        out[c * BPC:(c + 1) * BPC, :] = (
            arr.transpose(3, 0, 2, 1).reshape(BPC, OUT_DIM).astype(np.float32))
    return out


def kernel(x, weights, conn_a, conn_b) -> np.ndarray:
    nc = get_nc()
    in_maps = make_in_maps(x, weights, conn_a, conn_b)
    res = run_bass_kernel_spmd(nc, in_maps, core_ids=list(range(N_CORES)))
    return assemble_out(res.results)
